# revision 1
# baseline (speedup 1.0000x reference)
"""Multi-head self-attention prefill (B=2, S=2048, E=2048, H=16, D=128) on 8 trn2 cores.

Sharding: core c -> batch b = c//4, head-group g = c%4 (heads 4g..4g+3).
Each core computes q/k/v projections for its 4 heads (column shard of Wq/Wk/Wv),
causal attention with RoPE, and a partial output projection (row shard of Wo).
Host sums the 4 partials per batch (all-reduce equivalent) and stacks batches.

All matmuls run as float32r (reduced-precision fp32, full PE rate, ~1.5e-4 rel err).
"""
import sys
sys.path.insert(0, "/opt/trn_rl_repo")
import numpy as np

import concourse.bass as bass
import concourse.mybir as mybir
import concourse.tile as tile
from concourse import bacc
from concourse.bass import ds, ts
from concourse.masks import make_identity, make_causal_mask
from concourse.bass_utils import run_bass_kernel_spmd

S = 2048          # sequence length (per batch)
E = 2048          # embedding dim
H = 16            # total heads
D = 128           # head dim
HG = 4            # heads per core
DG = HG * D       # 512: per-core projection width
NE = E // 128     # 16 contraction chunks
NTB = 4           # token blocks of 512
TB = S // NTB     # 512
NTT = S // 128    # 16 token tiles of 128
NQC = 4           # q-chunks of 512
ROPE_BASE = 10000.0
MASK_VAL = -1e30

f32 = mybir.dt.float32
f32r = mybir.dt.float32r

_CACHE = {}


def build():
    nc = bacc.Bacc(None)
    x_in = nc.dram_tensor("x", [S, E], f32, kind="ExternalInput")
    wq_in = nc.dram_tensor("wq", [E, DG], f32, kind="ExternalInput")
    wk_in = nc.dram_tensor("wk", [E, DG], f32, kind="ExternalInput")
    wv_in = nc.dram_tensor("wv", [E, DG], f32, kind="ExternalInput")
    wo_in = nc.dram_tensor("wo", [DG, E], f32, kind="ExternalInput")
    cos_in = nc.dram_tensor("cosT", [128, S], f32, kind="ExternalInput")
    sin_in = nc.dram_tensor("sinT", [128, S], f32, kind="ExternalInput")
    out_d = nc.dram_tensor("out", [S, E], f32, kind="ExternalOutput")

    with tile.TileContext(nc) as tc:
        with tc.tile_pool(name="persist", bufs=1) as pp:
            # persistent across phases
            qT = [pp.tile([128, S], f32r, tag=f"qT{h}", name=f"qT{h}") for h in range(HG)]
            kT = [pp.tile([128, S], f32r, tag=f"kT{h}", name=f"kT{h}") for h in range(HG)]
            v_sb = [pp.tile([128, DG], f32r, tag=f"v{tt}", name=f"v{tt}") for tt in range(NTT)]
            ident = pp.tile([128, 128], f32r, tag="ident")
            identf = pp.tile([128, 128], f32, tag="identf")
            make_identity(nc, identf[:])
            nc.vector.tensor_copy(ident[:], identf[:])
            maskt = pp.tile([128, 128], f32, tag="mask")
            make_causal_mask(nc, maskt[:], mask_val=MASK_VAL)
            zero_r = pp.tile([128, 512], f32r, tag="zero_r")
            nc.gpsimd.memset(zero_r[:].bitcast(f32), 0.0)

            # ---------------- Phase A: x^T, projections, RoPE ----------------
            with tc.tile_pool(name="phA", bufs=1) as pa, \
                 tc.tile_pool(name="phA2", bufs=2) as pa2, \
                 tc.tile_pool(name="psA", bufs=2, space="PSUM") as psA:
                cosT = pa.tile([128, S], f32r, tag="cos")
                nc.sync.dma_start(out=cosT[:], in_=cos_in[:].bitcast(f32r))
                sinT = pa.tile([128, S], f32r, tag="sin")
                nc.sync.dma_start(out=sinT[:], in_=sin_in[:].bitcast(f32r))

                for tb in range(NTB):
                    # load x rows [tb*512, +512) in two half-blocks, transpose to xT[e] [128, 512]
                    xTs = [pa.tile([128, TB], f32r, tag=f"xT{e}", name=f"xT{e}")
                           for e in range(NE)]
                    for half in range(2):
                        xh = []
                        for t2 in range(2):
                            xt = pa.tile([128, E], f32r, tag=f"x{t2}")
                            r0 = tb * TB + half * 256 + t2 * 128
                            nc.sync.dma_start(out=xt[:], in_=x_in[r0:r0 + 128, :].bitcast(f32r))
                            xh.append(xt)
                        for e in range(NE):
                            ps = psA.tile([128, 256], f32r, tag="ptx")
                            for t2 in range(2):
                                nc.tensor.transpose(ps[:, ts(t2, 128)], xh[t2][:, ts(e, 128)], ident[:])
                            (nc.vector.tensor_copy if e % 2 else nc.scalar.copy)(
                                xTs[e][:, ds(half * 256, 256)], ps[:])

                    # q/k projections: stationary = W chunk, moving = xT
                    for w_idx, w_in in ((0, wq_in), (1, wk_in)):
                        dstT = qT if w_idx == 0 else kT
                        for dhp in range(2):
                            wts = []
                            for e in range(NE):
                                wt = pa2.tile([128, 256], f32r, tag=f"w{e}")
                                nc.sync.dma_start(
                                    out=wt[:],
                                    in_=w_in[ts(e, 128), ds(dhp * 256, 256)].bitcast(f32r))
                                wts.append(wt)
                            for dh2 in range(2):
                                h = dhp * 2 + dh2
                                ps = psA.tile([128, TB], f32, tag="pqk")
                                for e in range(NE):
                                    nc.tensor.matmul(ps[:], wts[e][:, ts(dh2, 128)], xTs[e][:],
                                                     start=(e == 0), stop=(e == NE - 1))
                                sl = dstT[h][:, ts(tb, TB)]
                                # RoPE via staging tile (decoupled dep chains):
                                # sl = stage*cos + swap(stage)*sin
                                stg = pa2.tile([128, TB], f32r, tag="stage")
                                nc.scalar.copy(stg[:], ps[:])
                                swp = pa2.tile([128, TB], f32r, tag="swap")
                                nc.sync.dma_start(out=swp[0:64, :], in_=stg[64:128, :])
                                nc.sync.dma_start(out=swp[64:128, :], in_=stg[0:64, :])
                                nc.vector.tensor_mul(swp[:], swp[:], sinT[:, ts(tb, TB)])
                                nc.vector.tensor_mul(sl, stg[:], cosT[:, ts(tb, TB)])
                                nc.vector.tensor_add(sl, sl, swp[:])
                    # v projection: stationary = xT chunk, moving = Wv chunk
                    for dhp in range(2):
                        wts = []
                        for e in range(NE):
                            wt = pa2.tile([128, 256], f32r, tag=f"w{e}")
                            nc.sync.dma_start(
                                out=wt[:],
                                in_=wv_in[ts(e, 128), ds(dhp * 256, 256)].bitcast(f32r))
                            wts.append(wt)
                        for t4 in range(4):
                            tt = tb * 4 + t4
                            ps = psA.tile([128, 256], f32, tag="pv")
                            for e in range(NE):
                                nc.tensor.matmul(ps[:], xTs[e][:, ts(t4, 128)], wts[e][:],
                                                 start=(e == 0), stop=(e == NE - 1))
                            nc.scalar.copy(v_sb[tt][:, ds(dhp * 256, 256)], ps[:])

            # ---------------- Phase B/C: attention + output projection ----------------
            with tc.tile_pool(name="phB", bufs=1) as pb, \
                 tc.tile_pool(name="phB2", bufs=2) as pb2, \
                 tc.tile_pool(name="phB3", bufs=3) as pb3, \
                 tc.tile_pool(name="psS", bufs=2, space="PSUM") as psS, \
                 tc.tile_pool(name="psT", bufs=2, space="PSUM") as psT, \
                 tc.tile_pool(name="psC", bufs=2, space="PSUM") as psC, \
                 tc.tile_pool(name="psO", bufs=2, space="PSUM") as psO:
                def softmax_block(qc, h):
                    """Scores + exp + normalize for the 4 q-tiles of chunk qc, head h."""
                    attn = {}
                    for q4 in range(4):
                        qt = 4 * qc + q4
                        q0 = qt * 128
                        nfull = q0 // 512
                        wstr = q0 + 128 - 512 * nfull  # straddle width
                        at = pb2.tile([128, S], f32r, tag=f"attn{q4}", name=f"attn{q4}")
                        attn[qt] = at
                        zsl = pb3.tile([128, 4], f32, tag="zsl", name="zsl", bufs=8)
                        for j in range(nfull + 1):
                            n = 512 if j < nfull else wstr
                            ps = psS.tile([128, 512], f32, tag="ps", name="ps")
                            nc.tensor.matmul(ps[:, 0:n], qT[h][:, ts(qt, 128)],
                                             kT[h][:, ds(512 * j, n)],
                                             start=True, stop=True)
                            if j == nfull:
                                nc.vector.tensor_add(ps[:, ds(n - 128, 128)],
                                                     ps[:, ds(n - 128, 128)], maskt[:])
                            nc.scalar.activation(at[:, ds(512 * j, n)], ps[:, 0:n],
                                                 mybir.ActivationFunctionType.Exp,
                                                 accum_out=zsl[:, j:j + 1])
                        z1 = pb3.tile([128, 1], f32, tag="z1", name="z1", bufs=8)
                        nc.vector.tensor_reduce(z1[:], zsl[:, 0:nfull + 1],
                                                axis=mybir.AxisListType.X,
                                                op=mybir.AluOpType.add)
                        rz = pb3.tile([128, 1], f32, tag="rz", name="rz", bufs=8)
                        nc.vector.reciprocal(rz[:], z1[:])
                        nc.vector.tensor_scalar_mul(at[:, 0:q0 + 128], at[:, 0:q0 + 128],
                                                    rz[:])
                    return attn

                # software pipeline: softmax for chunk qc+1 is queued before the
                # PE-heavy transpose/ctx work of chunk qc
                attn_store = {h: softmax_block(0, h) for h in range(HG)}
                for qc in range(NQC):
                    nkt = 4 * qc + 4        # k tiles needed for this q-chunk
                    next_attn = ({h: softmax_block(qc + 1, h) for h in range(HG)}
                                 if qc + 1 < NQC else None)
                    ctxT = {}
                    for h in range(HG):
                        attn = attn_store[h]
                        # --- transpose attn, multiply with v ---
                        pc = psC.tile([128, 512], f32, tag="pc", name="pc")
                        for kt in range(nkt):
                            pt = psT.tile([128, 512], f32r, tag="pt", name="pt")
                            zc = max(0, kt - 4 * qc)
                            for q4 in range(zc, 4):
                                qt = 4 * qc + q4
                                nc.tensor.transpose(pt[:, ts(q4, 128)],
                                                    attn[qt][:, ts(kt, 128)], ident[:])
                            aT = pb3.tile([128, 512], f32r, tag="aT", name="aT")
                            if zc > 0:
                                nc.vector.tensor_copy(aT[:, 0:zc * 128], zero_r[:, 0:zc * 128])
                            (nc.vector.tensor_copy if kt % 2 else nc.scalar.copy)(
                                aT[:, ds(zc * 128, 512 - zc * 128)],
                                pt[:, ds(zc * 128, 512 - zc * 128)])
                            nc.tensor.matmul(pc[:], v_sb[kt][:, ts(h, 128)], aT[:],
                                             start=(kt == 0), stop=(kt == nkt - 1))
                        ct = pb2.tile([128, 512], f32r, tag=f"ctxT{h}", name=f"ctxT{h}")
                        nc.vector.tensor_copy(ct[:], pc[:])
                        ctxT[h] = ct

                    # --- output projection for this q-chunk (wo streamed) ---
                    for e4 in range(4):
                        wos = []
                        for h in range(HG):
                            w = pb2.tile([128, 512], f32r, tag=f"wo{h}", name=f"wo{h}")
                            nc.sync.dma_start(out=w[:],
                                              in_=wo_in[ts(h, 128), ts(e4, 512)].bitcast(f32r))
                            wos.append(w)
                        for t4 in range(4):
                            row0 = qc * 512 + t4 * 128
                            po = psO.tile([128, 512], f32, tag="po", name="po")
                            for h in range(HG):
                                nc.tensor.matmul(po[:], ctxT[h][:, ts(t4, 128)],
                                                 wos[h][:],
                                                 start=(h == 0), stop=(h == HG - 1))
                            ob = pb3.tile([128, 512], f32, tag="ob", name="ob", bufs=2)
                            nc.scalar.copy(ob[:], po[:])
                            nc.sync.dma_start(out=out_d[row0:row0 + 128, ts(e4, 512)],
                                              in_=ob[:])
                    if next_attn is not None:
                        attn_store = next_attn
    nc.finalize()
    return nc


def _host_tables():
    half = D // 2
    inv = 1.0 / (ROPE_BASE ** (np.arange(half, dtype=np.float64) * 2.0 / D))
    ang = np.arange(S, dtype=np.float64)[None, :] * inv[:, None]   # [64, S]
    cos = np.cos(ang).astype(np.float32)
    sin = np.sin(ang).astype(np.float32)
    cosT = np.concatenate([cos, cos], axis=0)                      # [128, S]
    sinT = np.concatenate([-sin, sin], axis=0)                     # [128, S]
    return cosT, sinT


def kernel(x, start_pos, Wq, Wk, Wv, Wo):
    x = np.asarray(x, dtype=np.float32)
    Wq = np.asarray(Wq, dtype=np.float32)
    Wk = np.asarray(Wk, dtype=np.float32)
    Wv = np.asarray(Wv, dtype=np.float32)
    Wo = np.asarray(Wo, dtype=np.float32)
    B = x.shape[0]
    assert x.shape == (B, S, E) and B == 2

    cosT, sinT = _host_tables()
    perm = np.concatenate([np.arange(0, D, 2), np.arange(1, D, 2)])
    scale = 1.0 / np.sqrt(D)

    in_maps = []
    for c in range(8):
        b, g = c // 4, c % 4
        cols = slice(DG * g, DG * g + DG)
        wq = (Wq[:, cols] * scale).astype(np.float32).reshape(E, HG, D)[:, :, perm].reshape(E, DG)
        wk = Wk[:, cols].reshape(E, HG, D)[:, :, perm].reshape(E, DG)
        wv = np.ascontiguousarray(Wv[:, cols])
        wo = np.ascontiguousarray(Wo[cols, :])
        in_maps.append({
            "x": np.ascontiguousarray(x[b]),
            "wq": np.ascontiguousarray(wq),
            "wk": np.ascontiguousarray(wk),
            "wv": wv,
            "wo": wo,
            "cosT": cosT,
            "sinT": sinT,
        })

    if "nc" not in _CACHE:
        _CACHE["nc"] = build()
    nc = _CACHE["nc"]
    _CACHE["in_maps"] = in_maps
    res = run_bass_kernel_spmd(nc, in_maps, list(range(8)))
    parts = [res.results[c]["out"] for c in range(8)]
    out = np.stack([
        parts[0] + parts[1] + parts[2] + parts[3],
        parts[4] + parts[5] + parts[6] + parts[7],
    ]).astype(np.float32)
    return out



# revision 14
# speedup vs baseline: 1.6893x; 1.6893x over previous
"""Multi-head self-attention prefill (B=2, S=2048, E=2048, H=16, D=128) on 8 trn2 cores.

Sharding: core c -> batch b = c//4, head-group g = c%4 (heads 4g..4g+3).
Each core computes q/k/v projections for its 4 heads (column shard of Wq/Wk/Wv),
causal attention with RoPE, and a partial output projection (row shard of Wo).
Host sums the 4 partials per batch (all-reduce equivalent) and stacks batches.

v2: bf16 matmuls (fp32 PSUM accum), host-side x transpose + packed weight
layouts (single big DMAs, weights loaded once), 1024-wide moving operands,
causal-tight ctx accumulation, copies spread across scalar/vector engines.
"""
import sys
sys.path.insert(0, "/opt/trn_rl_repo")
import numpy as np
from ml_dtypes import bfloat16

import concourse.bass as bass
import concourse.mybir as mybir
import concourse.tile as tile
from concourse import bacc
from concourse.bass import ds, ts
from concourse.masks import make_identity, make_causal_mask
from concourse.bass_utils import run_bass_kernel_spmd

S = 2048          # sequence length (per batch)
E = 2048          # embedding dim
H = 16            # total heads
D = 128           # head dim
HG = 4            # heads per core
DG = HG * D       # 512: per-core projection width
NE = E // 128     # 16 contraction chunks
NTP = 2           # token super-blocks of 1024
TP = S // NTP     # 1024
NTT = S // 128    # 16 token tiles of 128
NQC = 4           # q-chunks of 512
ROPE_BASE = 10000.0
MASK_VAL = -1e30

f32 = mybir.dt.float32
bf16 = mybir.dt.bfloat16

_CACHE = {}
DEBUG = False


def build():
    nc = bacc.Bacc(None)
    # host-packed layouts (see kernel() for packing):
    #   xt:  [256, 16*1024]  xt[tp*128+p, e*1024+t] = x[tp*1024+t, e*128+p]
    #   wq/wk/wv: [128, 16*512]  w[p, e*512+d] = W[e*128+p, d]
    #   wo:  [128, 4*2048]   wo[p, h*2048+eo] = Wo[h*128+p, eo]
    xt_in = nc.dram_tensor("xt", [NTP * 128, NE * TP], bf16, kind="ExternalInput")
    wq_in = nc.dram_tensor("wq", [128, NE * DG], bf16, kind="ExternalInput")
    wk_in = nc.dram_tensor("wk", [128, NE * DG], bf16, kind="ExternalInput")
    wv_in = nc.dram_tensor("wv", [128, NE * DG], bf16, kind="ExternalInput")
    wo_in = nc.dram_tensor("wo", [128, HG * E], bf16, kind="ExternalInput")
    cos_in = nc.dram_tensor("cosT", [128, S], bf16, kind="ExternalInput")
    sin_in = nc.dram_tensor("sinT", [128, S], bf16, kind="ExternalInput")
    out_d = nc.dram_tensor("out", [S, E], bf16, kind="ExternalOutput")
    if DEBUG:
        dbg_q = nc.dram_tensor("dbg_q", [128, S], bf16, kind="ExternalOutput")
        dbg_k = nc.dram_tensor("dbg_k", [128, S], bf16, kind="ExternalOutput")
        dbg_v = nc.dram_tensor("dbg_v", [128, DG], bf16, kind="ExternalOutput")
        dbg_at = nc.dram_tensor("dbg_at", [128, S], bf16, kind="ExternalOutput")
        dbg_ct = nc.dram_tensor("dbg_ct", [128, DG], bf16, kind="ExternalOutput")
        dbg_a2 = nc.dram_tensor("dbg_a2", [128, 512], bf16, kind="ExternalOutput")
        dbg_a6 = nc.dram_tensor("dbg_a6", [128, 512], bf16, kind="ExternalOutput")

    with tile.TileContext(nc) as tc:
        with tc.tile_pool(name="persist", bufs=1) as pp:
            # persistent across phases
            qT = [pp.tile([128, S], bf16, tag=f"qT{h}", name=f"qT{h}") for h in range(HG)]
            kT = [pp.tile([128, S], bf16, tag=f"kT{h}", name=f"kT{h}") for h in range(HG)]
            v_sb = [pp.tile([128, DG], bf16, tag=f"v{tt}", name=f"v{tt}") for tt in range(NTT)]
            ident = pp.tile([128, 128], bf16, tag="ident")
            identf = pp.tile([128, 128], f32, tag="identf")
            make_identity(nc, identf[:])
            nc.vector.tensor_copy(ident[:], identf[:])
            maskt = pp.tile([128, 128], f32, tag="mask")
            make_causal_mask(nc, maskt[:], mask_val=MASK_VAL)
            zero_bf = pp.tile([128, 512], bf16, tag="zero_bf")
            nc.gpsimd.memset(zero_bf[:], 0.0)

            # ---------------- Phase A: projections + RoPE ----------------
            with tc.tile_pool(name="phA", bufs=1) as pa, \
                 tc.tile_pool(name="phA2", bufs=2) as pa2, \
                 tc.tile_pool(name="psQK", bufs=2, space="PSUM") as psQK, \
                 tc.tile_pool(name="psV", bufs=2, space="PSUM") as psV:
                # weights: one DMA each, split in 4 for queue parallelism
                wq_sb = pa.tile([128, NE * DG], bf16, tag="wq")
                wk_sb = pa.tile([128, NE * DG], bf16, tag="wk")
                wv_sb = pa.tile([128, NE * DG], bf16, tag="wv")
                for j in range(4):
                    nc.sync.dma_start(out=wq_sb[:, ts(j, 2048)], in_=wq_in[:, ts(j, 2048)])
                for j in range(4):
                    nc.sync.dma_start(out=wk_sb[:, ts(j, 2048)], in_=wk_in[:, ts(j, 2048)])
                cosT = pa.tile([128, S], bf16, tag="cos")
                nc.sync.dma_start(out=cosT[:], in_=cos_in[:])
                sinT = pa.tile([128, S], bf16, tag="sin")
                nc.sync.dma_start(out=sinT[:], in_=sin_in[:])
                for j in range(4):
                    nc.sync.dma_start(out=wv_sb[:, ts(j, 2048)], in_=wv_in[:, ts(j, 2048)])

                for tp in range(NTP):
                    xTs = pa2.tile([128, NE * TP], bf16, tag="xT", name="xT")
                    for j in range(4):
                        nc.sync.dma_start(
                            out=xTs[:, ts(j, NE * TP // 4)],
                            in_=xt_in[ds(tp * 128, 128), ts(j, NE * TP // 4)])

                    # q/k projections + RoPE (per head, 1024 tokens at a time;
                    # moving dim capped at 512 by the ISA -> two 512 chains)
                    for w_sb, dstT in ((wq_sb, qT), (wk_sb, kT)):
                        for h in range(HG):
                            ps = psQK.tile([128, TP], f32, tag="pqk", name="pqk")
                            for e in range(NE):
                                for hf in range(2):
                                    nc.tensor.matmul(
                                        ps[:, ts(hf, 512)],
                                        w_sb[:, ds(e * DG + h * 128, 128)],
                                        xTs[:, ds(e * TP + hf * 512, 512)],
                                        start=(e == 0), stop=(e == NE - 1))
                            sl = dstT[h][:, ts(tp, TP)]
                            cs = cosT[:, ts(tp, TP)]
                            sn = sinT[:, ts(tp, TP)]
                            # RoPE: sl = raw*cos + swap(raw)*sin  (sin signed +-)
                            nc.scalar.copy(sl, ps[:])
                            swp = pa2.tile([128, TP], bf16, tag="swp", name="swp")
                            nc.sync.dma_start(out=swp[0:64, :],
                                              in_=dstT[h][64:128, ts(tp, TP)])
                            nc.sync.dma_start(out=swp[64:128, :],
                                              in_=dstT[h][0:64, ts(tp, TP)])
                            nc.vector.tensor_mul(swp[:], swp[:], sn)
                            nc.vector.tensor_mul(sl, sl, cs)
                            nc.vector.tensor_add(sl, sl, swp[:])
                    # v projection: stationary = xT chunk, moving = Wv chunk
                    for t8 in range(8):
                        tt = tp * 8 + t8
                        ps = psV.tile([128, DG], f32, tag="pv", name="pv")
                        for e in range(NE):
                            nc.tensor.matmul(ps[:], xTs[:, ds(e * TP + t8 * 128, 128)],
                                             wv_sb[:, ts(e, DG)],
                                             start=(e == 0), stop=(e == NE - 1))
                        (nc.vector.tensor_copy if t8 % 2 else nc.scalar.copy)(
                            v_sb[tt][:], ps[:])

            if DEBUG:
                nc.sync.dma_start(out=dbg_q[:], in_=qT[0][:])
                nc.sync.dma_start(out=dbg_k[:], in_=kT[0][:])
                nc.sync.dma_start(out=dbg_v[:], in_=v_sb[4][:])

            # ---------------- Phase B: attention + output projection ----------------
            with tc.tile_pool(name="phB", bufs=1) as pb, \
                 tc.tile_pool(name="phB3", bufs=3) as pb3, \
                 tc.tile_pool(name="phB8", bufs=8) as pb8, \
                 tc.tile_pool(name="psS", bufs=2, space="PSUM") as psS, \
                 tc.tile_pool(name="psT", bufs=2, space="PSUM") as psT, \
                 tc.tile_pool(name="psCO", bufs=2, space="PSUM") as psCO:
                wo_sb = pb.tile([128, HG * E], bf16, tag="wo")
                for j in range(4):
                    nc.sync.dma_start(out=wo_sb[:, ts(j, 2048)], in_=wo_in[:, ts(j, 2048)])

                def softmax_block(qc, h):
                    """Scores + exp + normalize for the 4 q-tiles of chunk qc, head h."""
                    attn = {}
                    for q4 in range(4):
                        qt = 4 * qc + q4
                        row = (qt + 1) * 128          # causal row width
                        nj = (row + 1023) // 1024     # 1024-blocks
                        at = pb3.tile([128, S], bf16, tag=f"attn{q4}", name=f"attn{q4}")
                        attn[qt] = at
                        zsl = pb8.tile([128, 2], f32, tag="zsl", name="zsl")
                        for j in range(nj):
                            w = min(1024, row - 1024 * j)
                            ps = psS.tile([128, 1024], f32, tag="ps", name="ps")
                            for c0 in range(0, w, 512):
                                cw = min(512, w - c0)
                                nc.tensor.matmul(ps[:, ds(c0, cw)],
                                                 qT[h][:, ts(qt, 128)],
                                                 kT[h][:, ds(1024 * j + c0, cw)],
                                                 start=True, stop=True)
                            if j == nj - 1:
                                nc.vector.tensor_add(ps[:, ds(w - 128, 128)],
                                                     ps[:, ds(w - 128, 128)], maskt[:])
                            nc.scalar.activation(at[:, ds(1024 * j, w)], ps[:, 0:w],
                                                 mybir.ActivationFunctionType.Exp,
                                                 accum_out=zsl[:, j:j + 1])
                        rz = pb8.tile([128, 1], f32, tag="rz", name="rz")
                        if nj == 2:
                            z1 = pb8.tile([128, 1], f32, tag="z1", name="z1")
                            nc.vector.tensor_add(z1[:], zsl[:, 0:1], zsl[:, 1:2])
                            nc.vector.reciprocal(rz[:], z1[:])
                        else:
                            nc.vector.reciprocal(rz[:], zsl[:, 0:1])
                        nc.vector.tensor_scalar_mul(at[:, 0:row], at[:, 0:row], rz[:])
                        if DEBUG and qc == 1 and h == 0 and q4 == 1:
                            nc.sync.dma_start(out=dbg_at[:, 0:row], in_=at[:, 0:row])
                    return attn

                # software pipeline: softmax for (qc+1, h) queued right after the
                # PE-heavy transpose/ctx work of (qc, h)
                attn_store = {h: softmax_block(0, h) for h in range(HG)}
                ncp = 0  # round-robin for psum->sbuf copies
                for qc in range(NQC):
                    nkt = 4 * qc + 4        # k tiles needed for this q-chunk
                    next_attn = {}
                    ctxT = {}
                    for h in range(HG):
                        attn = attn_store[h]
                        # --- transpose attn, multiply with v (causal-tight) ---
                        pc = psCO.tile([128, 512], f32, tag="pco", name="pc")
                        for kt in range(nkt):
                            zc = max(0, kt - 4 * qc)
                            pt = psT.tile([128, 512], bf16, tag="pt", name="pt")
                            for q4 in range(zc, 4):
                                qt = 4 * qc + q4
                                nc.tensor.transpose(pt[:, ts(q4, 128)],
                                                    attn[qt][:, ts(kt, 128)], ident[:])
                            aT = pb3.tile([128, 512], bf16, tag="aT", name="aT", bufs=4)
                            wcp = 512 - zc * 128
                            if zc > 0:
                                nc.vector.tensor_copy(aT[:, 0:zc * 128],
                                                      zero_bf[:, 0:zc * 128])
                            (nc.vector.tensor_copy if ncp % 2 else nc.scalar.copy)(
                                aT[:, ds(zc * 128, wcp)], pt[:, ds(zc * 128, wcp)])
                            ncp += 1
                            if DEBUG and qc == 1 and h == 0 and kt in (2, 6):
                                dst = dbg_a2 if kt == 2 else dbg_a6
                                nc.sync.dma_start(out=dst[:, ds(zc * 128, wcp)],
                                                  in_=aT[:, ds(zc * 128, wcp)])
                            nc.tensor.matmul(pc[:], v_sb[kt][:, ts(h, 128)], aT[:],
                                             start=(kt == 0), stop=(kt == nkt - 1))
                        ct = pb3.tile([128, 512], bf16, tag=f"ctxT{h}", name=f"ctxT{h}", bufs=2)
                        nc.vector.tensor_copy(ct[:], pc[:])
                        ctxT[h] = ct
                        if DEBUG and qc == 1 and h == 0:
                            nc.sync.dma_start(out=dbg_ct[:], in_=ct[:])
                        if qc + 1 < NQC:
                            next_attn[h] = softmax_block(qc + 1, h)

                    # --- output projection for this q-chunk ---
                    for t4 in range(4):
                        row0 = qc * 512 + t4 * 128
                        ob = pb3.tile([128, E], bf16, tag="ob", name="ob", bufs=2)
                        for e4 in range(4):
                            po = psCO.tile([128, 512], f32, tag="pco", name="po")
                            for h in range(HG):
                                nc.tensor.matmul(po[:], ctxT[h][:, ts(t4, 128)],
                                                 wo_sb[:, ds(h * E + e4 * 512, 512)],
                                                 start=(h == 0), stop=(h == HG - 1))
                            (nc.vector.tensor_copy if e4 % 2 else nc.scalar.copy)(
                                ob[:, ts(e4, 512)], po[:])
                        nc.sync.dma_start(out=out_d[ds(row0, 128), :], in_=ob[:])
                    attn_store = next_attn
    nc.finalize()
    return nc


def _host_tables():
    half = D // 2
    inv = 1.0 / (ROPE_BASE ** (np.arange(half, dtype=np.float64) * 2.0 / D))
    ang = np.arange(S, dtype=np.float64)[None, :] * inv[:, None]   # [64, S]
    cos = np.cos(ang)
    sin = np.sin(ang)
    cosT = np.concatenate([cos, cos], axis=0)                      # [128, S]
    sinT = np.concatenate([-sin, sin], axis=0)                     # [128, S]
    return cosT.astype(bfloat16), sinT.astype(bfloat16)


def kernel(x, start_pos, Wq, Wk, Wv, Wo):
    x = np.asarray(x, dtype=np.float32)
    Wq = np.asarray(Wq, dtype=np.float32)
    Wk = np.asarray(Wk, dtype=np.float32)
    Wv = np.asarray(Wv, dtype=np.float32)
    Wo = np.asarray(Wo, dtype=np.float32)
    B = x.shape[0]
    assert x.shape == (B, S, E) and B == 2

    cosT, sinT = _host_tables()
    perm = np.concatenate([np.arange(0, D, 2), np.arange(1, D, 2)])
    scale = 1.0 / np.sqrt(D)

    def pack_w(w):  # [E, DG] -> [128, NE*DG]
        return np.ascontiguousarray(
            w.reshape(NE, 128, DG).transpose(1, 0, 2).reshape(128, NE * DG))

    in_maps = []
    for c in range(8):
        b, g = c // 4, c % 4
        cols = slice(DG * g, DG * g + DG)
        wq = (Wq[:, cols] * scale).reshape(E, HG, D)[:, :, perm].reshape(E, DG)
        wk = Wk[:, cols].reshape(E, HG, D)[:, :, perm].reshape(E, DG)
        wv = Wv[:, cols]
        # xt[tp*128+p, e*1024+t] = x[b, tp*1024+t, e*128+p]
        xt = (x[b].reshape(NTP, TP, NE, 128)      # [tp, t, e, p]
              .transpose(0, 3, 2, 1)              # [tp, p, e, t]
              .reshape(NTP * 128, NE * TP))
        # wo[p, h*2048+eo] = Wo[g*DG + h*128 + p, eo]
        wo = (Wo[cols, :].reshape(HG, 128, E)
              .transpose(1, 0, 2).reshape(128, HG * E))
        in_maps.append({
            "xt": np.ascontiguousarray(xt).astype(bfloat16),
            "wq": pack_w(wq).astype(bfloat16),
            "wk": pack_w(wk).astype(bfloat16),
            "wv": pack_w(wv).astype(bfloat16),
            "wo": np.ascontiguousarray(wo).astype(bfloat16),
            "cosT": cosT,
            "sinT": sinT,
        })

    if "nc" not in _CACHE:
        _CACHE["nc"] = build()
    nc = _CACHE["nc"]
    _CACHE["in_maps"] = in_maps
    res = run_bass_kernel_spmd(nc, in_maps, list(range(8)))
    parts = [res.results[c]["out"].astype(np.float32) for c in range(8)]
    out = np.stack([
        parts[0] + parts[1] + parts[2] + parts[3],
        parts[4] + parts[5] + parts[6] + parts[7],
    ]).astype(np.float32)
    return out


# revision 20
# speedup vs baseline: 1.8839x; 1.1152x over previous
"""Multi-head self-attention prefill (B=2, S=2048, E=2048, H=16, D=128) on 8 trn2 cores.

Sharding: core c -> batch b = c//4, head-group g = c%4 (heads 4g..4g+3).
Each core computes q/k/v projections for its 4 heads (column shard of Wq/Wk/Wv),
causal attention with RoPE, and a partial output projection (row shard of Wo).
Host sums the 4 partials per batch (all-reduce equivalent) and stacks batches.

v2: bf16 matmuls (fp32 PSUM accum), host-side x transpose + packed weight
layouts (single big DMAs, weights loaded once), 1024-wide moving operands,
causal-tight ctx accumulation, copies spread across scalar/vector engines.
"""
import sys
sys.path.insert(0, "/opt/trn_rl_repo")
import numpy as np
from ml_dtypes import bfloat16

import concourse.bass as bass
import concourse.mybir as mybir
import concourse.tile as tile
from concourse import bacc
from concourse.bass import ds, ts
from concourse.masks import make_identity, make_causal_mask
from concourse.bass_utils import run_bass_kernel_spmd

S = 2048          # sequence length (per batch)
E = 2048          # embedding dim
H = 16            # total heads
D = 128           # head dim
HG = 4            # heads per core
DG = HG * D       # 512: per-core projection width
NE = E // 128     # 16 contraction chunks
NTP = 2           # token super-blocks of 1024
TP = S // NTP     # 1024
NTT = S // 128    # 16 token tiles of 128
NQC = 4           # q-chunks of 512
ROPE_BASE = 10000.0
MASK_VAL = -1e30

f32 = mybir.dt.float32
bf16 = mybir.dt.bfloat16

_CACHE = {}
DEBUG = False


def build():
    nc = bacc.Bacc(None)
    # host-packed layouts (see kernel() for packing):
    #   xt:  [256, 16*1024]  xt[tp*128+p, e*1024+t] = x[tp*1024+t, e*128+p]
    #   wq/wk/wv: [128, 16*512]  w[p, e*512+d] = W[e*128+p, d]
    #   wo:  [128, 4*2048]   wo[p, h*2048+eo] = Wo[h*128+p, eo]
    xt_in = nc.dram_tensor("xt", [NTP * 128, NE * TP], bf16, kind="ExternalInput")
    wq_in = nc.dram_tensor("wq", [128, NE * DG], bf16, kind="ExternalInput")
    wk_in = nc.dram_tensor("wk", [128, NE * DG], bf16, kind="ExternalInput")
    wv_in = nc.dram_tensor("wv", [128, NE * DG], bf16, kind="ExternalInput")
    wo_in = nc.dram_tensor("wo", [128, HG * E], bf16, kind="ExternalInput")
    cos_in = nc.dram_tensor("cosT", [128, S], bf16, kind="ExternalInput")
    sin_in = nc.dram_tensor("sinT", [128, S], bf16, kind="ExternalInput")
    out_d = nc.dram_tensor("out", [S, E], bf16, kind="ExternalOutput")
    if DEBUG:
        dbg_q = nc.dram_tensor("dbg_q", [128, S], bf16, kind="ExternalOutput")
        dbg_k = nc.dram_tensor("dbg_k", [128, S], bf16, kind="ExternalOutput")
        dbg_v = nc.dram_tensor("dbg_v", [128, DG], bf16, kind="ExternalOutput")
        dbg_ct = nc.dram_tensor("dbg_ct", [128, DG], bf16, kind="ExternalOutput")
        dbg_et = nc.dram_tensor("dbg_et", [128, 1024], bf16, kind="ExternalOutput")

    with tile.TileContext(nc) as tc:
        with tc.tile_pool(name="persist", bufs=1) as pp:
            # persistent across phases
            qT = [pp.tile([128, S], bf16, tag=f"qT{h}", name=f"qT{h}") for h in range(HG)]
            kT = [pp.tile([128, S], bf16, tag=f"kT{h}", name=f"kT{h}") for h in range(HG)]
            v_sb = [pp.tile([128, DG], bf16, tag=f"v{tt}", name=f"v{tt}") for tt in range(NTT)]
            # transposed causal mask: maskTT[k, q] = 0 if q >= k else MASK_VAL
            maskTT = pp.tile([128, 128], f32, tag="maskTT")
            nc.gpsimd.memset(maskTT[:], 0.0)
            nc.gpsimd.affine_select(
                out=maskTT[:], in_=maskTT[:],
                compare_op=mybir.AluOpType.is_ge, fill=MASK_VAL,
                base=0, pattern=[[1, 128]], channel_multiplier=-1)
            zero_bf = pp.tile([128, 512], bf16, tag="zero_bf")
            nc.gpsimd.memset(zero_bf[:], 0.0)
            ones_sb = pp.tile([128, 128], bf16, tag="ones_sb")
            nc.gpsimd.memset(ones_sb[:], 1.0)

            # ---------------- Phase A: projections + RoPE ----------------
            with tc.tile_pool(name="phA", bufs=1) as pa, \
                 tc.tile_pool(name="phA2", bufs=2) as pa2, \
                 tc.tile_pool(name="psQK", bufs=2, space="PSUM") as psQK, \
                 tc.tile_pool(name="psV", bufs=2, space="PSUM") as psV:
                # weights + first x block, interleaved so the first q-proj
                # accumulation chain can start as soon as slices land
                wq_sb = pa.tile([128, NE * DG], bf16, tag="wq")
                wk_sb = pa.tile([128, NE * DG], bf16, tag="wk")
                wv_sb = pa.tile([128, NE * DG], bf16, tag="wv")
                xTs0 = pa2.tile([128, NE * TP], bf16, tag="xT", name="xTs0")
                for j in range(4):
                    nc.sync.dma_start(out=wq_sb[:, ts(j, 2048)], in_=wq_in[:, ts(j, 2048)])
                    nc.sync.dma_start(out=xTs0[:, ts(j, 4096)],
                                      in_=xt_in[0:128, ts(j, 4096)])
                for j in range(4):
                    nc.sync.dma_start(out=wk_sb[:, ts(j, 2048)], in_=wk_in[:, ts(j, 2048)])
                cosT = pa.tile([128, S], bf16, tag="cos")
                nc.sync.dma_start(out=cosT[:], in_=cos_in[:])
                sinT = pa.tile([128, S], bf16, tag="sin")
                nc.sync.dma_start(out=sinT[:], in_=sin_in[:])
                for j in range(4):
                    nc.sync.dma_start(out=wv_sb[:, ts(j, 2048)], in_=wv_in[:, ts(j, 2048)])

                for tp in range(NTP):
                    if tp == 0:
                        xTs = xTs0
                    else:
                        xTs = pa2.tile([128, NE * TP], bf16, tag="xT", name="xT")
                        for j in range(4):
                            nc.sync.dma_start(
                                out=xTs[:, ts(j, NE * TP // 4)],
                                in_=xt_in[ds(tp * 128, 128), ts(j, NE * TP // 4)])

                    # q/k projections + RoPE (per head, 1024 tokens at a time;
                    # moving dim capped at 512 by the ISA -> two 512 chains)
                    for w_sb, dstT in ((wq_sb, qT), (wk_sb, kT)):
                        for h in range(HG):
                            ps = psQK.tile([128, TP], f32, tag="pqk", name="pqk")
                            for e in range(NE):
                                for hf in range(2):
                                    nc.tensor.matmul(
                                        ps[:, ts(hf, 512)],
                                        w_sb[:, ds(e * DG + h * 128, 128)],
                                        xTs[:, ds(e * TP + hf * 512, 512)],
                                        start=(e == 0), stop=(e == NE - 1))
                            sl = dstT[h][:, ts(tp, TP)]
                            cs = cosT[:, ts(tp, TP)]
                            sn = sinT[:, ts(tp, TP)]
                            # RoPE: sl = raw*cos + swap(raw)*sin  (sin signed +-)
                            nc.scalar.copy(sl, ps[:])
                            swp = pa2.tile([128, TP], bf16, tag="swp", name="swp")
                            nc.sync.dma_start(out=swp[0:64, :],
                                              in_=dstT[h][64:128, ts(tp, TP)])
                            nc.sync.dma_start(out=swp[64:128, :],
                                              in_=dstT[h][0:64, ts(tp, TP)])
                            nc.vector.tensor_mul(swp[:], swp[:], sn)
                            nc.vector.tensor_mul(sl, sl, cs)
                            nc.vector.tensor_add(sl, sl, swp[:])
                    # v projection: stationary = xT chunk, moving = Wv chunk
                    for t8 in range(8):
                        tt = tp * 8 + t8
                        ps = psV.tile([128, DG], f32, tag="pv", name="pv")
                        for e in range(NE):
                            nc.tensor.matmul(ps[:], xTs[:, ds(e * TP + t8 * 128, 128)],
                                             wv_sb[:, ts(e, DG)],
                                             start=(e == 0), stop=(e == NE - 1))
                        (nc.vector.tensor_copy if t8 % 2 else nc.scalar.copy)(
                            v_sb[tt][:], ps[:])

            if DEBUG:
                nc.sync.dma_start(out=dbg_q[:], in_=qT[0][:])
                nc.sync.dma_start(out=dbg_k[:], in_=kT[0][:])
                nc.sync.dma_start(out=dbg_v[:], in_=v_sb[4][:])

            # ---------------- Phase B: attention + output projection ----------------
            # scores computed TRANSPOSED (S^T[k, q] via stationary=kT chunk), so no
            # PE transposes / PSUM->SBUF attn copies. z comes from an all-ones
            # stationary matmul (z replicated across partitions); normalization is
            # fused into the ctx PSUM->SBUF copy.
            with tc.tile_pool(name="phB", bufs=1) as pb, \
                 tc.tile_pool(name="phB3", bufs=3) as pb3, \
                 tc.tile_pool(name="psS", bufs=2, space="PSUM") as psS, \
                 tc.tile_pool(name="psZ", bufs=2, space="PSUM") as psZ, \
                 tc.tile_pool(name="psCO", bufs=2, space="PSUM") as psCO:
                wo_sb = pb.tile([128, HG * E], bf16, tag="wo")
                for j in range(4):
                    nc.sync.dma_start(out=wo_sb[:, ts(j, 2048)], in_=wo_in[:, ts(j, 2048)])

                ctxT = {}

                def st_pair(qc, h, k2):
                    """Emit S^T + mask + exp + zero-fill for kt pair; return et."""
                    q0 = qc * 512
                    pst = psS.tile([128, 1024], f32, tag="pst", name="pst")
                    et = pb3.tile([128, 1024], bf16, tag="et", name="et", bufs=4)
                    zcs = []
                    for kk in range(2):
                        kt = 2 * k2 + kk
                        zc = max(0, kt - 4 * qc)   # first valid q4
                        zcs.append(zc)
                        nc.tensor.matmul(
                            pst[:, ds(kk * 512 + zc * 128, 512 - zc * 128)],
                            kT[h][:, ts(kt, 128)],
                            qT[h][:, ds(q0 + zc * 128, 512 - zc * 128)],
                            start=True, stop=True)
                        if kt >= 4 * qc:  # diagonal tile: q4 == zc
                            sl = pst[:, ds(kk * 512 + zc * 128, 128)]
                            nc.vector.tensor_add(sl, sl, maskTT[:])
                    # exp over the whole pair (garbage spans overwritten below)
                    nc.scalar.activation(et[:], pst[:],
                                         mybir.ActivationFunctionType.Exp)
                    for kk in range(2):
                        if zcs[kk] > 0:
                            nc.vector.tensor_copy(
                                et[:, ds(kk * 512, zcs[kk] * 128)],
                                zero_bf[:, 0:zcs[kk] * 128])
                    if DEBUG and qc == 1 and h == 0 and k2 == 1:
                        nc.sync.dma_start(out=dbg_et[:], in_=et[:])
                    return et

                def consume(p):
                    """Emit z + ctx matmuls for a pending pair; epilogue on last."""
                    qc, h, k2, et, pc, zp, nkt = p
                    for kk in range(2):
                        kt = 2 * k2 + kk
                        nc.tensor.matmul(zp[:], ones_sb[:], et[:, ts(kk, 512)],
                                         start=(kt == 0), stop=(kt == nkt - 1))
                        nc.tensor.matmul(pc[:], v_sb[kt][:, ts(h, 128)],
                                         et[:, ts(kk, 512)],
                                         start=(kt == 0), stop=(kt == nkt - 1))
                    if k2 == nkt // 2 - 1:
                        rzb = pb3.tile([128, 512], f32, tag="rzb", name="rzb", bufs=2)
                        nc.vector.reciprocal(rzb[:], zp[:])
                        ct = pb3.tile([128, 512], bf16, tag=f"ctxT{h}",
                                      name=f"ctxT{h}", bufs=2)
                        nc.vector.tensor_mul(ct[:], pc[:], rzb[:])
                        ctxT[h] = ct
                        if DEBUG and qc == 1 and h == 0:
                            nc.sync.dma_start(out=dbg_ct[:], in_=ct[:])

                pending = None
                for qc in range(NQC):
                    nkt = 4 * qc + 4        # k tiles needed for this q-chunk
                    for h in range(HG):
                        pc = psCO.tile([128, 512], f32, tag="pco", name="pc")
                        zp = psZ.tile([128, 512], f32, tag="zp", name="zp")
                        for k2 in range(nkt // 2):
                            et = st_pair(qc, h, k2)
                            if pending is not None:
                                consume(pending)
                            pending = (qc, h, k2, et, pc, zp, nkt)
                    consume(pending)   # flush so all ctxT(qc, *) are emitted
                    pending = None

                    # --- output projection for this q-chunk ---
                    for t4 in range(4):
                        row0 = qc * 512 + t4 * 128
                        ob = pb3.tile([128, E], bf16, tag="ob", name="ob", bufs=2)
                        for e4 in range(4):
                            po = psCO.tile([128, 512], f32, tag="pco", name="po")
                            for h in range(HG):
                                nc.tensor.matmul(po[:], ctxT[h][:, ts(t4, 128)],
                                                 wo_sb[:, ds(h * E + e4 * 512, 512)],
                                                 start=(h == 0), stop=(h == HG - 1))
                            (nc.vector.tensor_copy if e4 % 2 else nc.scalar.copy)(
                                ob[:, ts(e4, 512)], po[:])
                        nc.sync.dma_start(out=out_d[ds(row0, 128), :], in_=ob[:])
    nc.finalize()
    return nc


def _host_tables():
    half = D // 2
    inv = 1.0 / (ROPE_BASE ** (np.arange(half, dtype=np.float64) * 2.0 / D))
    ang = np.arange(S, dtype=np.float64)[None, :] * inv[:, None]   # [64, S]
    cos = np.cos(ang)
    sin = np.sin(ang)
    cosT = np.concatenate([cos, cos], axis=0)                      # [128, S]
    sinT = np.concatenate([-sin, sin], axis=0)                     # [128, S]
    return cosT.astype(bfloat16), sinT.astype(bfloat16)


def kernel(x, start_pos, Wq, Wk, Wv, Wo):
    x = np.asarray(x, dtype=np.float32)
    Wq = np.asarray(Wq, dtype=np.float32)
    Wk = np.asarray(Wk, dtype=np.float32)
    Wv = np.asarray(Wv, dtype=np.float32)
    Wo = np.asarray(Wo, dtype=np.float32)
    B = x.shape[0]
    assert x.shape == (B, S, E) and B == 2

    cosT, sinT = _host_tables()
    perm = np.concatenate([np.arange(0, D, 2), np.arange(1, D, 2)])
    scale = 1.0 / np.sqrt(D)

    def pack_w(w):  # [E, DG] -> [128, NE*DG]
        return np.ascontiguousarray(
            w.reshape(NE, 128, DG).transpose(1, 0, 2).reshape(128, NE * DG))

    in_maps = []
    for c in range(8):
        b, g = c // 4, c % 4
        cols = slice(DG * g, DG * g + DG)
        wq = (Wq[:, cols] * scale).reshape(E, HG, D)[:, :, perm].reshape(E, DG)
        wk = Wk[:, cols].reshape(E, HG, D)[:, :, perm].reshape(E, DG)
        wv = Wv[:, cols]
        # xt[tp*128+p, e*1024+t] = x[b, tp*1024+t, e*128+p]
        xt = (x[b].reshape(NTP, TP, NE, 128)      # [tp, t, e, p]
              .transpose(0, 3, 2, 1)              # [tp, p, e, t]
              .reshape(NTP * 128, NE * TP))
        # wo[p, h*2048+eo] = Wo[g*DG + h*128 + p, eo]
        wo = (Wo[cols, :].reshape(HG, 128, E)
              .transpose(1, 0, 2).reshape(128, HG * E))
        in_maps.append({
            "xt": np.ascontiguousarray(xt).astype(bfloat16),
            "wq": pack_w(wq).astype(bfloat16),
            "wk": pack_w(wk).astype(bfloat16),
            "wv": pack_w(wv).astype(bfloat16),
            "wo": np.ascontiguousarray(wo).astype(bfloat16),
            "cosT": cosT,
            "sinT": sinT,
        })

    if "nc" not in _CACHE:
        _CACHE["nc"] = build()
    nc = _CACHE["nc"]
    _CACHE["in_maps"] = in_maps
    res = run_bass_kernel_spmd(nc, in_maps, list(range(8)))
    parts = [res.results[c]["out"].astype(np.float32) for c in range(8)]
    out = np.stack([
        parts[0] + parts[1] + parts[2] + parts[3],
        parts[4] + parts[5] + parts[6] + parts[7],
    ]).astype(np.float32)
    return out


# revision 23
# speedup vs baseline: 2.2054x; 1.1707x over previous
"""Multi-head self-attention prefill (B=2, S=2048, E=2048, H=16, D=128) on 8 trn2 cores.

Sharding: core c -> batch b = c//4, head-group g = c%4 (heads 4g..4g+3).
Each core computes q/k/v projections for its 4 heads (column shard of Wq/Wk/Wv),
causal attention with RoPE, and a partial output projection (row shard of Wo).
Host sums the 4 partials per batch (all-reduce equivalent) and stacks batches.

v2: bf16 matmuls (fp32 PSUM accum), host-side x transpose + packed weight
layouts (single big DMAs, weights loaded once), 1024-wide moving operands,
causal-tight ctx accumulation, copies spread across scalar/vector engines.
"""
import sys
sys.path.insert(0, "/opt/trn_rl_repo")
import numpy as np
from ml_dtypes import bfloat16

import concourse.bass as bass
import concourse.mybir as mybir
import concourse.tile as tile
from concourse import bacc
from concourse.bass import ds, ts
from concourse.masks import make_identity, make_causal_mask
from concourse.bass_utils import run_bass_kernel_spmd

S = 2048          # sequence length (per batch)
E = 2048          # embedding dim
H = 16            # total heads
D = 128           # head dim
HG = 4            # heads per core
DG = HG * D       # 512: per-core projection width
NE = E // 128     # 16 contraction chunks
NTP = 2           # token super-blocks of 1024
TP = S // NTP     # 1024
NTT = S // 128    # 16 token tiles of 128
NQC = 4           # q-chunks of 512
ROPE_BASE = 10000.0
MASK_VAL = -1e30

f32 = mybir.dt.float32
bf16 = mybir.dt.bfloat16

_CACHE = {}
DEBUG = False


def build():
    nc = bacc.Bacc(None)
    # host-packed layouts (see kernel() for packing):
    #   xt:  [256, 16*1024]  xt[tp*128+p, e*1024+t] = x[tp*1024+t, e*128+p]
    #   wq/wk/wv: [128, 16*512]  w[p, e*512+d] = W[e*128+p, d]
    #   wo:  [128, 4*2048]   wo[p, h*2048+eo] = Wo[h*128+p, eo]
    xt_in = nc.dram_tensor("xt", [NTP * 128, NE * TP], bf16, kind="ExternalInput")
    wq_in = nc.dram_tensor("wq", [128, NE * DG], bf16, kind="ExternalInput")
    wk_in = nc.dram_tensor("wk", [128, NE * DG], bf16, kind="ExternalInput")
    wv_in = nc.dram_tensor("wv", [128, NE * DG], bf16, kind="ExternalInput")
    wo_in = nc.dram_tensor("wo", [128, HG * E], bf16, kind="ExternalInput")
    cos_in = nc.dram_tensor("cosT", [128, S], bf16, kind="ExternalInput")
    sin_in = nc.dram_tensor("sinT", [128, S], bf16, kind="ExternalInput")
    out_d = nc.dram_tensor("out", [S, E], bf16, kind="ExternalOutput")
    if DEBUG:
        dbg_q = nc.dram_tensor("dbg_q", [128, S], bf16, kind="ExternalOutput")
        dbg_k = nc.dram_tensor("dbg_k", [128, S], bf16, kind="ExternalOutput")
        dbg_v = nc.dram_tensor("dbg_v", [128, DG], bf16, kind="ExternalOutput")
        dbg_ct = nc.dram_tensor("dbg_ct", [128, DG], bf16, kind="ExternalOutput")
        dbg_et = nc.dram_tensor("dbg_et", [128, 1024], bf16, kind="ExternalOutput")

    with tile.TileContext(nc) as tc:
        with tc.tile_pool(name="persist", bufs=1) as pp:
            # persistent across phases
            qT = [pp.tile([128, S], bf16, tag=f"qT{h}", name=f"qT{h}") for h in range(HG)]
            kT = [pp.tile([128, S], bf16, tag=f"kT{h}", name=f"kT{h}") for h in range(HG)]
            v_sb = [pp.tile([128, DG], bf16, tag=f"v{tt}", name=f"v{tt}") for tt in range(NTT)]
            # transposed causal mask: maskTT[k, q] = 0 if q >= k else MASK_VAL
            maskTT = pp.tile([128, 128], f32, tag="maskTT")
            nc.gpsimd.memset(maskTT[:], 0.0)
            nc.gpsimd.affine_select(
                out=maskTT[:], in_=maskTT[:],
                compare_op=mybir.AluOpType.is_ge, fill=MASK_VAL,
                base=0, pattern=[[1, 128]], channel_multiplier=-1)
            zero_bf = pp.tile([128, 512], bf16, tag="zero_bf")
            nc.gpsimd.memset(zero_bf[:], 0.0)
            ones_sb = pp.tile([128, 128], bf16, tag="ones_sb")
            nc.gpsimd.memset(ones_sb[:], 1.0)

            # ---------------- Phase A: projections + RoPE ----------------
            with tc.tile_pool(name="phA", bufs=1) as pa, \
                 tc.tile_pool(name="phA2", bufs=2) as pa2, \
                 tc.tile_pool(name="psQK", bufs=2, space="PSUM") as psQK, \
                 tc.tile_pool(name="psV", bufs=2, space="PSUM") as psV:
                # weights + first x block, interleaved so the first q-proj
                # accumulation chain can start as soon as slices land
                wq_sb = pa.tile([128, NE * DG], bf16, tag="wq")
                wk_sb = pa.tile([128, NE * DG], bf16, tag="wk")
                wv_sb = pa.tile([128, NE * DG], bf16, tag="wv")
                xTs0 = pa2.tile([128, NE * TP], bf16, tag="xT", name="xTs0")
                for j in range(4):
                    nc.sync.dma_start(out=wq_sb[:, ts(j, 2048)], in_=wq_in[:, ts(j, 2048)])
                    nc.sync.dma_start(out=xTs0[:, ts(j, 4096)],
                                      in_=xt_in[0:128, ts(j, 4096)])
                for j in range(4):
                    nc.sync.dma_start(out=wk_sb[:, ts(j, 2048)], in_=wk_in[:, ts(j, 2048)])
                cosT = pa.tile([128, S], bf16, tag="cos")
                nc.sync.dma_start(out=cosT[:], in_=cos_in[:])
                sinT = pa.tile([128, S], bf16, tag="sin")
                nc.sync.dma_start(out=sinT[:], in_=sin_in[:])
                for j in range(4):
                    nc.sync.dma_start(out=wv_sb[:, ts(j, 2048)], in_=wv_in[:, ts(j, 2048)])

                for tp in range(NTP):
                    if tp == 0:
                        xTs = xTs0
                    else:
                        xTs = pa2.tile([128, NE * TP], bf16, tag="xT", name="xT")
                        for j in range(4):
                            nc.sync.dma_start(
                                out=xTs[:, ts(j, NE * TP // 4)],
                                in_=xt_in[ds(tp * 128, 128), ts(j, NE * TP // 4)])

                    # q/k projections + RoPE (per head, 1024 tokens at a time;
                    # moving dim capped at 512 by the ISA -> two 512 chains)
                    for w_sb, dstT in ((wq_sb, qT), (wk_sb, kT)):
                        for h in range(HG):
                            ps = psQK.tile([128, TP], f32, tag="pqk", name="pqk")
                            for e in range(NE):
                                for hf in range(2):
                                    nc.tensor.matmul(
                                        ps[:, ts(hf, 512)],
                                        w_sb[:, ds(e * DG + h * 128, 128)],
                                        xTs[:, ds(e * TP + hf * 512, 512)],
                                        start=(e == 0), stop=(e == NE - 1))
                            sl = dstT[h][:, ts(tp, TP)]
                            cs = cosT[:, ts(tp, TP)]
                            sn = sinT[:, ts(tp, TP)]
                            # RoPE: sl = raw*cos + swap(raw)*sin  (sin signed +-)
                            nc.scalar.copy(sl, ps[:])
                            swp = pa2.tile([128, TP], bf16, tag="swp", name="swp")
                            nc.sync.dma_start(out=swp[0:64, :],
                                              in_=dstT[h][64:128, ts(tp, TP)])
                            nc.sync.dma_start(out=swp[64:128, :],
                                              in_=dstT[h][0:64, ts(tp, TP)])
                            nc.vector.tensor_mul(swp[:], swp[:], sn)
                            nc.vector.tensor_mul(sl, sl, cs)
                            nc.vector.tensor_add(sl, sl, swp[:])
                    # v projection: stationary = xT chunk, moving = Wv chunk
                    for t8 in range(8):
                        tt = tp * 8 + t8
                        ps = psV.tile([128, DG], f32, tag="pv", name="pv")
                        for e in range(NE):
                            nc.tensor.matmul(ps[:], xTs[:, ds(e * TP + t8 * 128, 128)],
                                             wv_sb[:, ts(e, DG)],
                                             start=(e == 0), stop=(e == NE - 1))
                        (nc.vector.tensor_copy if t8 % 2 else nc.scalar.copy)(
                            v_sb[tt][:], ps[:])

            if DEBUG:
                nc.sync.dma_start(out=dbg_q[:], in_=qT[0][:])
                nc.sync.dma_start(out=dbg_k[:], in_=kT[0][:])
                nc.sync.dma_start(out=dbg_v[:], in_=v_sb[4][:])

            # ---------------- Phase B: attention + output projection ----------------
            # scores computed TRANSPOSED (S^T[k, q] via stationary=kT chunk), so no
            # PE transposes / PSUM->SBUF attn copies. z comes from an all-ones
            # stationary matmul (z replicated across partitions); normalization is
            # fused into the ctx PSUM->SBUF copy.
            with tc.tile_pool(name="phB", bufs=1) as pb, \
                 tc.tile_pool(name="phB3", bufs=3) as pb3, \
                 tc.tile_pool(name="psS", bufs=2, space="PSUM") as psS, \
                 tc.tile_pool(name="psZ", bufs=2, space="PSUM") as psZ, \
                 tc.tile_pool(name="psCO", bufs=2, space="PSUM") as psCO:
                wo_sb = pb.tile([128, HG * E], bf16, tag="wo")
                for j in range(4):
                    nc.sync.dma_start(out=wo_sb[:, ts(j, 2048)], in_=wo_in[:, ts(j, 2048)])

                ctxT = {}

                def st_one(qc, h, kt):
                    """Emit S^T + mask + exp + zero-fill for one kt; return et."""
                    zc = max(0, kt - 4 * qc)   # first valid q4
                    pst = psS.tile([128, 512], f32, tag="pst", name="pst", bufs=4)
                    et = pb3.tile([128, 512], bf16, tag="et", name="et", bufs=6)
                    w = 512 - zc * 128
                    nc.tensor.matmul(pst[:, ds(zc * 128, w)],
                                     kT[h][:, ts(kt, 128)],
                                     qT[h][:, ds(qc * 512 + zc * 128, w)],
                                     start=True, stop=True)
                    if kt >= 4 * qc:  # diagonal tile: q4 == zc
                        sl = pst[:, ds(zc * 128, 128)]
                        nc.vector.tensor_add(sl, sl, maskTT[:])
                    nc.scalar.activation(et[:, ds(zc * 128, w)],
                                         pst[:, ds(zc * 128, w)],
                                         mybir.ActivationFunctionType.Exp)
                    if zc > 0:
                        nc.vector.tensor_copy(et[:, 0:zc * 128],
                                              zero_bf[:, 0:zc * 128])
                    if DEBUG and qc == 1 and h == 0 and kt in (2, 3):
                        nc.sync.dma_start(out=dbg_et[:, ts(kt - 2, 512)], in_=et[:])
                    return et

                def consume(p):
                    """Emit z + ctx matmuls for a pending kt; epilogue on last."""
                    qc, h, kt, et, pc, zp, nkt = p
                    nc.tensor.matmul(zp[:], ones_sb[:], et[:],
                                     start=(kt == 0), stop=(kt == nkt - 1))
                    nc.tensor.matmul(pc[:], v_sb[kt][:, ts(h, 128)], et[:],
                                     start=(kt == 0), stop=(kt == nkt - 1))
                    if kt == nkt - 1:
                        rzb = pb3.tile([128, 512], f32, tag="rzb", name="rzb", bufs=2)
                        nc.vector.reciprocal_approx_fast(rzb[:], zp[:])
                        ct = pb3.tile([128, 512], bf16, tag=f"ctxT{h}",
                                      name=f"ctxT{h}", bufs=2)
                        nc.vector.tensor_mul(ct[:], pc[:], rzb[:])
                        ctxT[(qc, h)] = ct
                        if DEBUG and qc == 1 and h == 0:
                            nc.sync.dma_start(out=dbg_ct[:], in_=ct[:])

                def emit_op(qc):
                    """Output projection for q-chunk qc (needs ctxT[(qc, 0..3)])."""
                    for t4 in range(4):
                        row0 = qc * 512 + t4 * 128
                        ob = pb3.tile([128, E], bf16, tag="ob", name="ob", bufs=2)
                        for e4 in range(4):
                            po = psCO.tile([128, 512], f32, tag="pco", name="po")
                            for h in range(HG):
                                nc.tensor.matmul(po[:],
                                                 ctxT[(qc, h)][:, ts(t4, 128)],
                                                 wo_sb[:, ds(h * E + e4 * 512, 512)],
                                                 start=(h == 0), stop=(h == HG - 1))
                            (nc.vector.tensor_copy if e4 % 2 else nc.scalar.copy)(
                                ob[:, ts(e4, 512)], po[:])
                        nc.sync.dma_start(out=out_d[ds(row0, 128), :], in_=ob[:])

                # pipeline: z/ctx consumption lags S^T/exp by LAG kts, carrying
                # across (qc, h) blocks; OP(qc) is emitted a few kts into
                # (qc+1, h0) so the last epilogue's DVE latency is hidden.
                LAG = 3
                from collections import deque
                pending = deque()
                op_queue = None
                for qc in range(NQC):
                    nkt = 4 * qc + 4        # k tiles needed for this q-chunk
                    for h in range(HG):
                        pc = psCO.tile([128, 512], f32, tag="pco", name="pc")
                        zp = psZ.tile([128, 512], f32, tag="zp", name="zp")
                        for kt in range(nkt):
                            et = st_one(qc, h, kt)
                            pending.append((qc, h, kt, et, pc, zp, nkt))
                            if len(pending) > LAG:
                                consume(pending.popleft())
                            if op_queue is not None and h == 0 and kt == 4:
                                emit_op(op_queue)
                                op_queue = None
                    op_queue = qc
                while pending:
                    consume(pending.popleft())
                emit_op(NQC - 1)
    nc.finalize()
    return nc


def _host_tables():
    half = D // 2
    inv = 1.0 / (ROPE_BASE ** (np.arange(half, dtype=np.float64) * 2.0 / D))
    ang = np.arange(S, dtype=np.float64)[None, :] * inv[:, None]   # [64, S]
    cos = np.cos(ang)
    sin = np.sin(ang)
    cosT = np.concatenate([cos, cos], axis=0)                      # [128, S]
    sinT = np.concatenate([-sin, sin], axis=0)                     # [128, S]
    return cosT.astype(bfloat16), sinT.astype(bfloat16)


def kernel(x, start_pos, Wq, Wk, Wv, Wo):
    x = np.asarray(x, dtype=np.float32)
    Wq = np.asarray(Wq, dtype=np.float32)
    Wk = np.asarray(Wk, dtype=np.float32)
    Wv = np.asarray(Wv, dtype=np.float32)
    Wo = np.asarray(Wo, dtype=np.float32)
    B = x.shape[0]
    assert x.shape == (B, S, E) and B == 2

    cosT, sinT = _host_tables()
    perm = np.concatenate([np.arange(0, D, 2), np.arange(1, D, 2)])
    scale = 1.0 / np.sqrt(D)

    def pack_w(w):  # [E, DG] -> [128, NE*DG]
        return np.ascontiguousarray(
            w.reshape(NE, 128, DG).transpose(1, 0, 2).reshape(128, NE * DG))

    in_maps = []
    for c in range(8):
        b, g = c // 4, c % 4
        cols = slice(DG * g, DG * g + DG)
        wq = (Wq[:, cols] * scale).reshape(E, HG, D)[:, :, perm].reshape(E, DG)
        wk = Wk[:, cols].reshape(E, HG, D)[:, :, perm].reshape(E, DG)
        wv = Wv[:, cols]
        # xt[tp*128+p, e*1024+t] = x[b, tp*1024+t, e*128+p]
        xt = (x[b].reshape(NTP, TP, NE, 128)      # [tp, t, e, p]
              .transpose(0, 3, 2, 1)              # [tp, p, e, t]
              .reshape(NTP * 128, NE * TP))
        # wo[p, h*2048+eo] = Wo[g*DG + h*128 + p, eo]
        wo = (Wo[cols, :].reshape(HG, 128, E)
              .transpose(1, 0, 2).reshape(128, HG * E))
        in_maps.append({
            "xt": np.ascontiguousarray(xt).astype(bfloat16),
            "wq": pack_w(wq).astype(bfloat16),
            "wk": pack_w(wk).astype(bfloat16),
            "wv": pack_w(wv).astype(bfloat16),
            "wo": np.ascontiguousarray(wo).astype(bfloat16),
            "cosT": cosT,
            "sinT": sinT,
        })

    if "nc" not in _CACHE:
        _CACHE["nc"] = build()
    nc = _CACHE["nc"]
    _CACHE["in_maps"] = in_maps
    res = run_bass_kernel_spmd(nc, in_maps, list(range(8)))
    parts = [res.results[c]["out"].astype(np.float32) for c in range(8)]
    out = np.stack([
        parts[0] + parts[1] + parts[2] + parts[3],
        parts[4] + parts[5] + parts[6] + parts[7],
    ]).astype(np.float32)
    return out


# revision 25
# speedup vs baseline: 2.2420x; 1.0166x over previous
"""Multi-head self-attention prefill (B=2, S=2048, E=2048, H=16, D=128) on 8 trn2 cores.

Sharding: core c -> batch b = c//4, head-group g = c%4 (heads 4g..4g+3).
Each core computes q/k/v projections for its 4 heads (column shard of Wq/Wk/Wv),
causal attention with RoPE, and a partial output projection (row shard of Wo).
Host sums the 4 partials per batch (all-reduce equivalent) and stacks batches.

v2: bf16 matmuls (fp32 PSUM accum), host-side x transpose + packed weight
layouts (single big DMAs, weights loaded once), 1024-wide moving operands,
causal-tight ctx accumulation, copies spread across scalar/vector engines.
"""
import sys
sys.path.insert(0, "/opt/trn_rl_repo")
import numpy as np
from ml_dtypes import bfloat16

import concourse.bass as bass
import concourse.mybir as mybir
import concourse.tile as tile
from concourse import bacc
from concourse.bass import ds, ts
from concourse.masks import make_identity, make_causal_mask
from concourse.bass_utils import run_bass_kernel_spmd

S = 2048          # sequence length (per batch)
E = 2048          # embedding dim
H = 16            # total heads
D = 128           # head dim
HG = 4            # heads per core
DG = HG * D       # 512: per-core projection width
NE = E // 128     # 16 contraction chunks
NTP = 2           # token super-blocks of 1024
TP = S // NTP     # 1024
NTT = S // 128    # 16 token tiles of 128
NQC = 4           # q-chunks of 512
ROPE_BASE = 10000.0
MASK_VAL = -1e30

f32 = mybir.dt.float32
bf16 = mybir.dt.bfloat16

_CACHE = {}
DEBUG = False


def build():
    nc = bacc.Bacc(None)
    # host-packed layouts (see kernel() for packing):
    #   xt:  [256, 16*1024]  xt[tp*128+p, e*1024+t] = x[tp*1024+t, e*128+p]
    #   wq/wk/wv: [128, 16*512]  w[p, e*512+d] = W[e*128+p, d]
    #   wo:  [128, 4*2048]   wo[p, h*2048+eo] = Wo[h*128+p, eo]
    xt_in = nc.dram_tensor("xt", [NTP * 128, NE * TP], bf16, kind="ExternalInput")
    wq_in = nc.dram_tensor("wq", [128, NE * DG], bf16, kind="ExternalInput")
    wk_in = nc.dram_tensor("wk", [128, NE * DG], bf16, kind="ExternalInput")
    wv_in = nc.dram_tensor("wv", [128, NE * DG], bf16, kind="ExternalInput")
    wo_in = nc.dram_tensor("wo", [128, HG * E], bf16, kind="ExternalInput")
    cos_in = nc.dram_tensor("cosT", [128, S], bf16, kind="ExternalInput")
    sin_in = nc.dram_tensor("sinT", [128, S], bf16, kind="ExternalInput")
    out_d = nc.dram_tensor("out", [S, E], bf16, kind="ExternalOutput")
    if DEBUG:
        dbg_q = nc.dram_tensor("dbg_q", [128, S], bf16, kind="ExternalOutput")
        dbg_k = nc.dram_tensor("dbg_k", [128, S], bf16, kind="ExternalOutput")
        dbg_v = nc.dram_tensor("dbg_v", [128, DG], bf16, kind="ExternalOutput")
        dbg_ct = nc.dram_tensor("dbg_ct", [128, DG], bf16, kind="ExternalOutput")
        dbg_et = nc.dram_tensor("dbg_et", [128, 1024], bf16, kind="ExternalOutput")

    with tile.TileContext(nc) as tc:
        with tc.tile_pool(name="persist", bufs=1) as pp:
            # persistent across phases
            qT = [pp.tile([128, S], bf16, tag=f"qT{h}", name=f"qT{h}") for h in range(HG)]
            kT = [pp.tile([128, S], bf16, tag=f"kT{h}", name=f"kT{h}") for h in range(HG)]
            v_sb = [pp.tile([128, DG], bf16, tag=f"v{tt}", name=f"v{tt}") for tt in range(NTT)]
            # transposed causal mask: maskTT[k, q] = 0 if q >= k else MASK_VAL
            maskTT = pp.tile([128, 128], f32, tag="maskTT")
            nc.gpsimd.memset(maskTT[:], 0.0)
            nc.gpsimd.affine_select(
                out=maskTT[:], in_=maskTT[:],
                compare_op=mybir.AluOpType.is_ge, fill=MASK_VAL,
                base=0, pattern=[[1, 128]], channel_multiplier=-1)
            zero_bf = pp.tile([128, 512], bf16, tag="zero_bf")
            nc.gpsimd.memset(zero_bf[:], 0.0)
            ones_sb = pp.tile([128, 128], bf16, tag="ones_sb")
            nc.gpsimd.memset(ones_sb[:], 1.0)

            # ---------------- Phase A: projections + RoPE ----------------
            with tc.tile_pool(name="phA", bufs=1) as pa, \
                 tc.tile_pool(name="phA2", bufs=2) as pa2, \
                 tc.tile_pool(name="psQK", bufs=2, space="PSUM") as psQK, \
                 tc.tile_pool(name="psV", bufs=2, space="PSUM") as psV:
                # weights + first x block, interleaved so the first q-proj
                # accumulation chain can start as soon as slices land
                wq_sb = pa.tile([128, NE * DG], bf16, tag="wq")
                wk_sb = pa.tile([128, NE * DG], bf16, tag="wk")
                wv_sb = pa.tile([128, NE * DG], bf16, tag="wv")
                xTs0 = pa2.tile([128, NE * TP], bf16, tag="xT", name="xTs0")
                for j in range(8):
                    nc.sync.dma_start(out=wq_sb[:, ts(j, 1024)], in_=wq_in[:, ts(j, 1024)])
                    nc.sync.dma_start(out=xTs0[:, ts(j, 2048)],
                                      in_=xt_in[0:128, ts(j, 2048)])
                for j in range(4):
                    nc.sync.dma_start(out=wk_sb[:, ts(j, 2048)], in_=wk_in[:, ts(j, 2048)])
                cosT = pa.tile([128, S], bf16, tag="cos")
                nc.sync.dma_start(out=cosT[:], in_=cos_in[:])
                sinT = pa.tile([128, S], bf16, tag="sin")
                nc.sync.dma_start(out=sinT[:], in_=sin_in[:])
                for j in range(4):
                    nc.sync.dma_start(out=wv_sb[:, ts(j, 2048)], in_=wv_in[:, ts(j, 2048)])

                for tp in range(NTP):
                    if tp == 0:
                        xTs = xTs0
                    else:
                        xTs = pa2.tile([128, NE * TP], bf16, tag="xT", name="xT")
                        for j in range(4):
                            nc.sync.dma_start(
                                out=xTs[:, ts(j, NE * TP // 4)],
                                in_=xt_in[ds(tp * 128, 128), ts(j, NE * TP // 4)])

                    # q/k projections + RoPE (per head, 1024 tokens at a time;
                    # moving dim capped at 512 by the ISA -> two 512 chains)
                    for w_sb, dstT in ((wq_sb, qT), (wk_sb, kT)):
                        for h in range(HG):
                            ps = psQK.tile([128, TP], f32, tag="pqk", name="pqk")
                            for e in range(NE):
                                for hf in range(2):
                                    nc.tensor.matmul(
                                        ps[:, ts(hf, 512)],
                                        w_sb[:, ds(e * DG + h * 128, 128)],
                                        xTs[:, ds(e * TP + hf * 512, 512)],
                                        start=(e == 0), stop=(e == NE - 1))
                            sl = dstT[h][:, ts(tp, TP)]
                            cs = cosT[:, ts(tp, TP)]
                            sn = sinT[:, ts(tp, TP)]
                            # RoPE: sl = raw*cos + swap(raw)*sin  (sin signed +-)
                            nc.scalar.copy(sl, ps[:])
                            swp = pa2.tile([128, TP], bf16, tag="swp", name="swp")
                            nc.sync.dma_start(out=swp[0:64, :],
                                              in_=dstT[h][64:128, ts(tp, TP)])
                            nc.sync.dma_start(out=swp[64:128, :],
                                              in_=dstT[h][0:64, ts(tp, TP)])
                            nc.vector.tensor_mul(swp[:], swp[:], sn)
                            nc.vector.tensor_mul(sl, sl, cs)
                            nc.vector.tensor_add(sl, sl, swp[:])
                    # v projection: stationary = xT chunk, moving = Wv chunk
                    for t8 in range(8):
                        tt = tp * 8 + t8
                        ps = psV.tile([128, DG], f32, tag="pv", name="pv")
                        for e in range(NE):
                            nc.tensor.matmul(ps[:], xTs[:, ds(e * TP + t8 * 128, 128)],
                                             wv_sb[:, ts(e, DG)],
                                             start=(e == 0), stop=(e == NE - 1))
                        (nc.vector.tensor_copy if t8 % 2 else nc.scalar.copy)(
                            v_sb[tt][:], ps[:])

            if DEBUG:
                nc.sync.dma_start(out=dbg_q[:], in_=qT[0][:])
                nc.sync.dma_start(out=dbg_k[:], in_=kT[0][:])
                nc.sync.dma_start(out=dbg_v[:], in_=v_sb[4][:])

            # ---------------- Phase B: attention + output projection ----------------
            # scores computed TRANSPOSED (S^T[k, q] via stationary=kT chunk), so no
            # PE transposes / PSUM->SBUF attn copies. z comes from an all-ones
            # stationary matmul (z replicated across partitions); normalization is
            # fused into the ctx PSUM->SBUF copy.
            with tc.tile_pool(name="phB", bufs=1) as pb, \
                 tc.tile_pool(name="phB3", bufs=3) as pb3, \
                 tc.tile_pool(name="psS", bufs=2, space="PSUM") as psS, \
                 tc.tile_pool(name="psZ", bufs=2, space="PSUM") as psZ, \
                 tc.tile_pool(name="psCO", bufs=2, space="PSUM") as psCO:
                wo_sb = pb.tile([128, HG * E], bf16, tag="wo")
                for j in range(4):
                    nc.sync.dma_start(out=wo_sb[:, ts(j, 2048)], in_=wo_in[:, ts(j, 2048)])

                ctxT = {}

                def st_one(qc, h, kt):
                    """Emit S^T + mask + exp + zero-fill for one kt; return et."""
                    zc = max(0, kt - 4 * qc)   # first valid q4
                    pst = psS.tile([128, 512], f32, tag="pst", name="pst", bufs=4)
                    et = pb3.tile([128, 512], bf16, tag="et", name="et", bufs=6)
                    w = 512 - zc * 128
                    nc.tensor.matmul(pst[:, ds(zc * 128, w)],
                                     kT[h][:, ts(kt, 128)],
                                     qT[h][:, ds(qc * 512 + zc * 128, w)],
                                     start=True, stop=True)
                    if kt >= 4 * qc:  # diagonal tile: q4 == zc
                        sl = pst[:, ds(zc * 128, 128)]
                        nc.vector.tensor_add(sl, sl, maskTT[:])
                    nc.scalar.activation(et[:, ds(zc * 128, w)],
                                         pst[:, ds(zc * 128, w)],
                                         mybir.ActivationFunctionType.Exp)
                    if zc > 0:
                        nc.vector.tensor_copy(et[:, 0:zc * 128],
                                              zero_bf[:, 0:zc * 128])
                    if DEBUG and qc == 1 and h == 0 and kt in (2, 3):
                        nc.sync.dma_start(out=dbg_et[:, ts(kt - 2, 512)], in_=et[:])
                    return et

                last_et = {}

                def consume(p):
                    """Emit z + ctx matmuls for a pending kt; epilogue on last.

                    z is accumulated per kt-PAIR: the two et tiles are summed on
                    DVE first, halving the z matmul count. ctx matmuls read only
                    the causally-valid span (zero-filled spans feed only z)."""
                    qc, h, kt, et, pc, zp, nkt = p
                    if kt % 2 == 0:
                        last_et[(qc, h)] = et
                    else:
                        es = pb3.tile([128, 512], bf16, tag="es", name="es", bufs=3)
                        nc.vector.tensor_add(es[:], last_et[(qc, h)][:], et[:])
                        nc.tensor.matmul(zp[:], ones_sb[:], es[:],
                                         start=(kt == 1), stop=(kt == nkt - 1))
                    zc = max(0, kt - 4 * qc)
                    nc.tensor.matmul(pc[:, ds(zc * 128, 512 - zc * 128)],
                                     v_sb[kt][:, ts(h, 128)],
                                     et[:, ds(zc * 128, 512 - zc * 128)],
                                     start=(kt == 0), stop=(kt == nkt - 1),
                                     skip_group_check=(zc > 0))
                    if kt == nkt - 1:
                        rzb = pb3.tile([128, 512], f32, tag="rzb", name="rzb", bufs=2)
                        nc.vector.reciprocal_approx_fast(rzb[:], zp[:])
                        ct = pb3.tile([128, 512], bf16, tag=f"ctxT{h}",
                                      name=f"ctxT{h}", bufs=2)
                        nc.vector.tensor_mul(ct[:], pc[:], rzb[:])
                        ctxT[(qc, h)] = ct
                        if DEBUG and qc == 1 and h == 0:
                            nc.sync.dma_start(out=dbg_ct[:], in_=ct[:])

                def emit_op(qc):
                    """Output projection for q-chunk qc (needs ctxT[(qc, 0..3)])."""
                    for t4 in range(4):
                        row0 = qc * 512 + t4 * 128
                        ob = pb3.tile([128, E], bf16, tag="ob", name="ob", bufs=2)
                        for e4 in range(4):
                            po = psCO.tile([128, 512], f32, tag="pco", name="po")
                            for h in range(HG):
                                nc.tensor.matmul(po[:],
                                                 ctxT[(qc, h)][:, ts(t4, 128)],
                                                 wo_sb[:, ds(h * E + e4 * 512, 512)],
                                                 start=(h == 0), stop=(h == HG - 1))
                            (nc.vector.tensor_copy if e4 % 2 else nc.scalar.copy)(
                                ob[:, ts(e4, 512)], po[:])
                        nc.sync.dma_start(out=out_d[ds(row0, 128), :], in_=ob[:])

                # pipeline: z/ctx consumption lags S^T/exp by LAG kts, carrying
                # across (qc, h) blocks; OP(qc) is emitted a few kts into
                # (qc+1, h0) so the last epilogue's DVE latency is hidden.
                LAG = 3
                from collections import deque
                pending = deque()
                op_queue = None
                for qc in range(NQC):
                    nkt = 4 * qc + 4        # k tiles needed for this q-chunk
                    for h in range(HG):
                        pc = psCO.tile([128, 512], f32, tag="pco", name="pc")
                        zp = psZ.tile([128, 512], f32, tag="zp", name="zp")
                        for kt in range(nkt):
                            et = st_one(qc, h, kt)
                            pending.append((qc, h, kt, et, pc, zp, nkt))
                            if len(pending) > LAG:
                                consume(pending.popleft())
                            if op_queue is not None and h == 0 and kt == 4:
                                emit_op(op_queue)
                                op_queue = None
                    op_queue = qc
                while pending:
                    consume(pending.popleft())
                emit_op(NQC - 1)
    nc.finalize()
    return nc


def _host_tables():
    half = D // 2
    inv = 1.0 / (ROPE_BASE ** (np.arange(half, dtype=np.float64) * 2.0 / D))
    ang = np.arange(S, dtype=np.float64)[None, :] * inv[:, None]   # [64, S]
    cos = np.cos(ang)
    sin = np.sin(ang)
    cosT = np.concatenate([cos, cos], axis=0)                      # [128, S]
    sinT = np.concatenate([-sin, sin], axis=0)                     # [128, S]
    return cosT.astype(bfloat16), sinT.astype(bfloat16)


def kernel(x, start_pos, Wq, Wk, Wv, Wo):
    x = np.asarray(x, dtype=np.float32)
    Wq = np.asarray(Wq, dtype=np.float32)
    Wk = np.asarray(Wk, dtype=np.float32)
    Wv = np.asarray(Wv, dtype=np.float32)
    Wo = np.asarray(Wo, dtype=np.float32)
    B = x.shape[0]
    assert x.shape == (B, S, E) and B == 2

    cosT, sinT = _host_tables()
    perm = np.concatenate([np.arange(0, D, 2), np.arange(1, D, 2)])
    scale = 1.0 / np.sqrt(D)

    def pack_w(w):  # [E, DG] -> [128, NE*DG]
        return np.ascontiguousarray(
            w.reshape(NE, 128, DG).transpose(1, 0, 2).reshape(128, NE * DG))

    in_maps = []
    for c in range(8):
        b, g = c // 4, c % 4
        cols = slice(DG * g, DG * g + DG)
        wq = (Wq[:, cols] * scale).reshape(E, HG, D)[:, :, perm].reshape(E, DG)
        wk = Wk[:, cols].reshape(E, HG, D)[:, :, perm].reshape(E, DG)
        wv = Wv[:, cols]
        # xt[tp*128+p, e*1024+t] = x[b, tp*1024+t, e*128+p]
        xt = (x[b].reshape(NTP, TP, NE, 128)      # [tp, t, e, p]
              .transpose(0, 3, 2, 1)              # [tp, p, e, t]
              .reshape(NTP * 128, NE * TP))
        # wo[p, h*2048+eo] = Wo[g*DG + h*128 + p, eo]
        wo = (Wo[cols, :].reshape(HG, 128, E)
              .transpose(1, 0, 2).reshape(128, HG * E))
        in_maps.append({
            "xt": np.ascontiguousarray(xt).astype(bfloat16),
            "wq": pack_w(wq).astype(bfloat16),
            "wk": pack_w(wk).astype(bfloat16),
            "wv": pack_w(wv).astype(bfloat16),
            "wo": np.ascontiguousarray(wo).astype(bfloat16),
            "cosT": cosT,
            "sinT": sinT,
        })

    if "nc" not in _CACHE:
        _CACHE["nc"] = build()
    nc = _CACHE["nc"]
    _CACHE["in_maps"] = in_maps
    res = run_bass_kernel_spmd(nc, in_maps, list(range(8)))
    parts = [res.results[c]["out"].astype(np.float32) for c in range(8)]
    out = np.stack([
        parts[0] + parts[1] + parts[2] + parts[3],
        parts[4] + parts[5] + parts[6] + parts[7],
    ]).astype(np.float32)
    return out
